# revision 7
# baseline (speedup 1.0000x reference)
"""Causal multi-head attention (B=8, S=1024, E=1024, H=16, D=64) on 8 TRN2 NeuronCores.

Strategy: pure data parallelism over the batch — one batch element per core,
full weights replicated, zero collectives. Per-core flash-style attention:

  - host passes x[b] pre-transposed (xT = [E, S]) and weights reshaped/cast to
    bf16; all matmuls run in bf16 with fp32 PSUM accumulation.
  - QT/KT = [head*64+d, s] computed with W chunks stationary, xT moving;
    V = [s, head*64+d] with xT stationary, Wv moving.
  - scores[q,k] per (head, q-tile) with causal tile skipping; the diagonal
    128x128 block gets a -1e9 causal mask added on DVE before exp.
  - softmax skips the max-subtraction (scores ~ N(0,1), exp is safe) and gets
    the row sum for free via the ACT accum_out of the exp pass.
  - attn is transposed on the PE by a matmul against diag(1/rowsum) — this
    folds the softmax normalization into the transpose for free.
  - attn@V accumulates outT = [d, q] per head with V stationary, which lands
    directly in the CT = [head*64+d, s] layout the output projection needs.
  - out = CT.T @ Wo + bo, bias added on DVE from a host-broadcast bo tile.
"""

import numpy as np
import ml_dtypes

B, S, E = 8, 1024, 1024
H, D = 16, 64
HD = H * D
NCORES = 8
P = 128
NCH = E // P  # 8 contraction chunks
NT = S // P  # 8 q tiles
SCALE = 1.0 / np.sqrt(D)
BF16 = ml_dtypes.bfloat16

_graph_cache = {}


def _patch_tile_drain():
    """The walrus build in this container only allows a single sync wait on the
    TPB_CTRL Drain that TileContext emits at kernel tail. Spread the end-of-
    kernel waits across SP nops (one wait each) before the drain instead."""
    import concourse.tile as tile
    import concourse.mybir as mybir
    from concourse.vector_clock import ScopedClock

    if getattr(tile.TileContext, "_drain_patched", False):
        return

    def _drain_and_barrier(self, tick_clock, wait_clock):
        nop0 = self.nc.sync.nop(nofuse=True)
        wait_clock.add_sem_waits(
            nop0.ins, ScopedClock({None: tick_clock.global_clock})
        )
        waits = list(nop0.ins.sync_info.on_wait) if nop0.ins.sync_info else []
        if len(waits) > 1:
            nop0.ins.sync_info = mybir.SyncInfo(
                on_wait=waits[:1], on_update=list(nop0.ins.sync_info.on_update)
            )
            for w in waits[1:]:
                n = self.nc.sync.nop(nofuse=True)
                n.ins.sync_info = mybir.SyncInfo(on_wait=[w], on_update=[])
        self.nc.sync.drain()
        self.nc.all_engine_barrier()
        assert self.sems is not None
        popped = self.nc._tile_sem_poison_stack.pop()
        assert popped is self._sem_poison
        self.nc.clear_and_free_semaphores(list(self.sems.allocated().values()))
        self.nc.all_engine_barrier()

    tile.TileContext._drain_and_barrier = _drain_and_barrier
    tile.TileContext._drain_patched = True


def _split_waits(nc, maxw=1):
    """This container's walrus build rejects instructions carrying more than
    one sync wait. Move excess waits onto same-engine NoOps inserted directly
    before the instruction (sequencer order makes this semantically identical).
    Safe for a straight-line Tile program: waits only reference predecessors
    in the dependency DAG, so stalling the sequencer earlier cannot deadlock."""
    import concourse.mybir as mybir

    ctr = [0]
    for fn in nc.m.functions:
        for bb in fn.blocks:
            insts = bb.instructions
            out = []
            changed = False
            for inst in insts:
                si = getattr(inst, "sync_info", None)
                if si is not None and len(si.on_wait) > maxw:
                    waits = list(si.on_wait)
                    keep = waits[:maxw]
                    rest = waits[maxw:]
                    for i in range(0, len(rest), maxw):
                        n = mybir.InstNoOp(
                            name=f"waitsplit-{ctr[0]}", ins=[], outs=[]
                        )
                        ctr[0] += 1
                        n.engine = inst.engine
                        n.sync_info = mybir.SyncInfo(
                            on_wait=rest[i : i + maxw], on_update=[]
                        )
                        out.append(n)
                    inst.sync_info = mybir.SyncInfo(
                        on_wait=keep, on_update=list(si.on_update)
                    )
                    changed = True
                out.append(inst)
            if changed:
                bb.instructions = out
    return nc


def _build():
    import concourse.bass as bass
    import concourse.tile as tile
    import concourse.mybir as mybir
    from concourse.masks import make_identity, make_causal_mask

    f32 = mybir.dt.float32
    bf16 = mybir.dt.bfloat16

    nc = bass.Bass("TRN2", target_bir_lowering=False, debug=False, num_devices=NCORES)
    xT_ext = nc.declare_dram_parameter("xT", [E, S], bf16, isOutput=False)
    wq_ext = nc.declare_dram_parameter("wq", [E, HD], bf16, isOutput=False)
    wk_ext = nc.declare_dram_parameter("wk", [E, HD], bf16, isOutput=False)
    wv_ext = nc.declare_dram_parameter("wv", [E, HD], bf16, isOutput=False)
    wo_ext = nc.declare_dram_parameter("wo", [HD, E], bf16, isOutput=False)
    bo_ext = nc.declare_dram_parameter("bo", [P, E], f32, isOutput=False)
    out_ext = nc.declare_dram_parameter("out", [S, E], f32, isOutput=True)

    EXP = mybir.ActivationFunctionType.Exp

    with tile.TileContext(nc) as tc:
        with (
            tc.tile_pool(name="consts", bufs=1) as consts,
            tc.tile_pool(name="qt", bufs=NCH) as qt_pool,
            tc.tile_pool(name="kt", bufs=NCH) as kt_pool,
            tc.tile_pool(name="vp", bufs=NT) as v_pool,
            tc.tile_pool(name="ct", bufs=NCH) as ct_pool,
            tc.tile_pool(name="wo", bufs=NCH) as wo_pool,
            tc.tile_pool(name="outp", bufs=2) as out_pool,
            tc.tile_pool(name="sums", bufs=2) as sums_pool,
        ):
            identity = consts.tile([P, P], bf16, tag="ident")
            make_identity(nc, identity[:])
            mask = consts.tile([P, P], f32, tag="mask")
            make_causal_mask(nc, mask[:], mask_val=-1e9)
            bo_sb = consts.tile([P, E], f32, tag="bo")
            nc.sync.dma_start(out=bo_sb[:], in_=bo_ext[:])

            wo_sb = [wo_pool.tile([P, E], bf16, tag="wo", name=f"wo{_}") for _ in range(NCH)]
            for c in range(NCH):
                nc.sync.dma_start(out=wo_sb[c][:], in_=wo_ext[c * P : (c + 1) * P, :])

            qt_sb = [qt_pool.tile([P, S], bf16, tag="qt", name=f"qt{_}") for _ in range(NCH)]
            kt_sb = [kt_pool.tile([P, S], bf16, tag="kt", name=f"kt{_}") for _ in range(NCH)]
            v_sb = [v_pool.tile([P, HD], bf16, tag="v", name=f"v{_}") for _ in range(NT)]
            ct_sb = [ct_pool.tile([P, S], bf16, tag="ct", name=f"ct{_}") for _ in range(NCH)]

            # ---------- phase 1: QKV projections ----------
            with (
                tc.tile_pool(name="xt", bufs=NCH) as xt_pool,
                tc.tile_pool(name="w3", bufs=3 * NCH) as w3_pool,
                tc.tile_pool(name="psum1", bufs=3, space="PSUM") as psum1,
            ):
                xt_sb = [xt_pool.tile([P, S], bf16, tag="xt", name=f"xt{_}") for _ in range(NCH)]
                wq_sb = [w3_pool.tile([P, HD], bf16, tag="w3", name=f"wq{_}") for _ in range(NCH)]
                wk_sb = [w3_pool.tile([P, HD], bf16, tag="w3", name=f"wk{_}") for _ in range(NCH)]
                wv_sb = [w3_pool.tile([P, HD], bf16, tag="w3", name=f"wv{_}") for _ in range(NCH)]
                for c in range(NCH):
                    rows = slice(c * P, (c + 1) * P)
                    nc.sync.dma_start(out=xt_sb[c][:], in_=xT_ext[rows, :])
                    nc.sync.dma_start(out=wq_sb[c][:], in_=wq_ext[rows, :])
                    nc.sync.dma_start(out=wk_sb[c][:], in_=wk_ext[rows, :])
                    nc.sync.dma_start(out=wv_sb[c][:], in_=wv_ext[rows, :])

                for w_sb, dst in ((wq_sb, qt_sb), (wk_sb, kt_sb)):
                    for c in range(NCH):
                        ps = psum1.tile([P, S], f32, tag="mm")
                        for k in range(NCH):
                            for hf in range(2):
                                nc.tensor.matmul(
                                    ps[:, hf * 512 : (hf + 1) * 512],
                                    lhsT=w_sb[k][:, c * P : (c + 1) * P],
                                    rhs=xt_sb[k][:, hf * 512 : (hf + 1) * 512],
                                    start=(k == 0),
                                    stop=(k == NCH - 1),
                                )
                        nc.scalar.copy(out=dst[c][:], in_=ps[:])

                for t in range(NT):
                    ps = psum1.tile([P, HD], f32, tag="mm")
                    for k in range(NCH):
                        for hf in range(2):
                            nc.tensor.matmul(
                                ps[:, hf * 512 : (hf + 1) * 512],
                                lhsT=xt_sb[k][:, t * P : (t + 1) * P],
                                rhs=wv_sb[k][:, hf * 512 : (hf + 1) * 512],
                                start=(k == 0),
                                stop=(k == NCH - 1),
                            )
                    nc.scalar.copy(out=v_sb[t][:], in_=ps[:])

            # ---------- phase 2: attention per head ----------
            with (
                tc.tile_pool(name="attn", bufs=3) as attn_pool,
                tc.tile_pool(name="attnT", bufs=2) as attnT_pool,
                tc.tile_pool(name="diag", bufs=3) as diag_pool,
                tc.tile_pool(name="psum_sc", bufs=2, space="PSUM") as psum_sc,
                tc.tile_pool(name="psum_pt", bufs=2, space="PSUM") as psum_pt,
                tc.tile_pool(name="psum_ot", bufs=1, space="PSUM") as psum_ot,
            ):
                for h in range(H):
                    cq = h // 2
                    off = (h % 2) * D
                    qt_h = qt_sb[cq][off : off + D, :]
                    kt_h = kt_sb[cq][off : off + D, :]
                    sums_h = sums_pool.tile([P, NT], f32, tag="sums")
                    r_h = sums_pool.tile([P, NT], f32, tag="r")
                    attnT_tiles = []
                    for t in range(NT):
                        k_len = (t + 1) * P
                        sc = psum_sc.tile([P, S], f32, tag="sc")
                        for j in range((k_len + 511) // 512):
                            c0, c1 = j * 512, min(k_len, (j + 1) * 512)
                            nc.tensor.matmul(
                                sc[:, c0:c1],
                                lhsT=qt_h[:, t * P : (t + 1) * P],
                                rhs=kt_h[:, c0:c1],
                                start=True,
                                stop=True,
                            )
                        nc.vector.tensor_add(
                            sc[:, t * P : k_len], sc[:, t * P : k_len], mask[:]
                        )
                        attn = attn_pool.tile([P, S], bf16, tag="attn")
                        nc.scalar.activation(
                            attn[:, :k_len],
                            sc[:, :k_len],
                            EXP,
                            bias=0.0,
                            scale=float(SCALE),
                            accum_out=sums_h[:, t : t + 1],
                        )
                        nc.vector.reciprocal(r_h[:, t : t + 1], sums_h[:, t : t + 1])
                        dg = diag_pool.tile([P, P], bf16, tag="diag")
                        nc.gpsimd.tensor_scalar_mul(
                            dg[:], identity[:], r_h[:, t : t + 1]
                        )
                        aT = attnT_pool.tile([P, S], bf16, tag=f"aT{t}")
                        attnT_tiles.append(aT)
                        for g0 in range(0, t + 1, 4):
                            g1 = min(g0 + 4, t + 1)
                            pt = psum_pt.tile([P, 512], f32, tag="pt")
                            for kc in range(g0, g1):
                                nc.tensor.matmul(
                                    pt[:, (kc - g0) * P : (kc - g0 + 1) * P],
                                    lhsT=attn[:, kc * P : (kc + 1) * P],
                                    rhs=dg[:],
                                    start=True,
                                    stop=True,
                                )
                            nc.vector.tensor_copy(
                                aT[:, g0 * P : g1 * P], pt[:, : (g1 - g0) * P]
                            )
                    # t-outer so each PSUM accumulation group is contiguous:
                    # a start=True matmul invalidates has_written for its whole
                    # bank, so groups sharing a bank must not interleave.
                    ot = psum_ot.tile([D, S], f32, tag="ot")
                    for t in range(NT):
                        for kc in range(t + 1):
                            nc.tensor.matmul(
                                ot[:, t * P : (t + 1) * P],
                                lhsT=v_sb[kc][:, h * D : (h + 1) * D],
                                rhs=attnT_tiles[t][:, kc * P : (kc + 1) * P],
                                start=(kc == 0),
                                stop=(kc == t),
                            )
                    nc.scalar.copy(out=ct_sb[cq][off : off + D, :], in_=ot[:])

            # ---------- phase 3: output projection + bias ----------
            with tc.tile_pool(name="psum3", bufs=2, space="PSUM") as psum3:
                for t in range(NT):
                    op = psum3.tile([P, E], f32, tag="mm3")
                    for c in range(NCH):
                        for hf in range(2):
                            nc.tensor.matmul(
                                op[:, hf * 512 : (hf + 1) * 512],
                                lhsT=ct_sb[c][:, t * P : (t + 1) * P],
                                rhs=wo_sb[c][:, hf * 512 : (hf + 1) * 512],
                                start=(c == 0),
                                stop=(c == NCH - 1),
                            )
                    osb = out_pool.tile([P, E], f32, tag="out")
                    nc.vector.tensor_add(osb[:], op[:], bo_sb[:])
                    nc.sync.dma_start(
                        out=out_ext[t * P : (t + 1) * P, :], in_=osb[:]
                    )

    return _split_waits(nc)


def _get_graph():
    if "nc" not in _graph_cache:
        _patch_tile_drain()
        _graph_cache["nc"] = _build()
    return _graph_cache["nc"]


def _prep_inputs(x, Wq, Wk, Wv, Wo, bo):
    xT = np.ascontiguousarray(np.transpose(np.asarray(x, np.float32), (0, 2, 1)))
    xT = xT.astype(BF16)
    wq = np.ascontiguousarray(
        np.asarray(Wq, np.float32).transpose(1, 0, 2).reshape(E, HD)
    ).astype(BF16)
    wk = np.ascontiguousarray(
        np.asarray(Wk, np.float32).transpose(1, 0, 2).reshape(E, HD)
    ).astype(BF16)
    wv = np.ascontiguousarray(
        np.asarray(Wv, np.float32).transpose(1, 0, 2).reshape(E, HD)
    ).astype(BF16)
    wo = np.ascontiguousarray(np.asarray(Wo, np.float32)).astype(BF16)
    bo_t = np.ascontiguousarray(
        np.tile(np.asarray(bo, np.float32)[None, :], (P, 1))
    )
    return [
        dict(
            xT=np.ascontiguousarray(xT[b]),
            wq=wq,
            wk=wk,
            wv=wv,
            wo=wo,
            bo=bo_t,
        )
        for b in range(B)
    ]


def _run(in_maps, **kw):
    from concourse.bass_utils import run_bass_kernel_spmd

    nc = _get_graph()
    return run_bass_kernel_spmd(nc, in_maps, core_ids=list(range(NCORES)), **kw)


def kernel(x, Wq, Wk, Wv, Wo, bo):
    res = _run(_prep_inputs(x, Wq, Wk, Wv, Wo, bo))
    return np.stack(
        [np.asarray(res.results[b]["out"], np.float32) for b in range(B)], axis=0
    )


# revision 8
# speedup vs baseline: 1.4041x; 1.4041x over previous
"""Causal multi-head attention (B=8, S=1024, E=1024, H=16, D=64) on 8 TRN2 NeuronCores.

Strategy: pure data parallelism over the batch — one batch element per core,
full weights replicated, zero collectives. Per-core flash-style attention:

  - host passes x[b] pre-transposed (xT = [E, S]) and weights reshaped/cast to
    bf16; all matmuls run in bf16 with fp32 PSUM accumulation.
  - QT/KT = [head*64+d, s] computed with W chunks stationary, xT moving;
    V = [s, head*64+d] with xT stationary, Wv moving.
  - scores[q,k] per (head, q-tile) with causal tile skipping; the diagonal
    128x128 block gets a -1e9 causal mask added on DVE before exp.
  - softmax skips the max-subtraction (scores ~ N(0,1), exp is safe) and gets
    the row sum for free via the ACT accum_out of the exp pass.
  - attn is transposed on the PE by a matmul against diag(1/rowsum) — this
    folds the softmax normalization into the transpose for free.
  - attn@V accumulates outT = [d, q] per head with V stationary, which lands
    directly in the CT = [head*64+d, s] layout the output projection needs.
  - out = CT.T @ Wo + bo, bias added on DVE from a host-broadcast bo tile.
"""

import numpy as np
import ml_dtypes

B, S, E = 8, 1024, 1024
H, D = 16, 64
HD = H * D
NCORES = 8
P = 128
NCH = E // P  # 8 contraction chunks
NT = S // P  # 8 q tiles
SCALE = 1.0 / np.sqrt(D)
BF16 = ml_dtypes.bfloat16

_graph_cache = {}


def _patch_tile_drain():
    """The walrus build in this container only allows a single sync wait on the
    TPB_CTRL Drain that TileContext emits at kernel tail. Spread the end-of-
    kernel waits across SP nops (one wait each) before the drain instead."""
    import concourse.tile as tile
    import concourse.mybir as mybir
    from concourse.vector_clock import ScopedClock

    if getattr(tile.TileContext, "_drain_patched", False):
        return

    def _drain_and_barrier(self, tick_clock, wait_clock):
        nop0 = self.nc.sync.nop(nofuse=True)
        wait_clock.add_sem_waits(
            nop0.ins, ScopedClock({None: tick_clock.global_clock})
        )
        waits = list(nop0.ins.sync_info.on_wait) if nop0.ins.sync_info else []
        if len(waits) > 1:
            nop0.ins.sync_info = mybir.SyncInfo(
                on_wait=waits[:1], on_update=list(nop0.ins.sync_info.on_update)
            )
            for w in waits[1:]:
                n = self.nc.sync.nop(nofuse=True)
                n.ins.sync_info = mybir.SyncInfo(on_wait=[w], on_update=[])
        self.nc.sync.drain()
        self.nc.all_engine_barrier()
        assert self.sems is not None
        popped = self.nc._tile_sem_poison_stack.pop()
        assert popped is self._sem_poison
        self.nc.clear_and_free_semaphores(list(self.sems.allocated().values()))
        self.nc.all_engine_barrier()

    tile.TileContext._drain_and_barrier = _drain_and_barrier
    tile.TileContext._drain_patched = True


def _split_waits(nc, maxw=1):
    """This container's walrus build rejects instructions carrying more than
    one sync wait. Move excess waits onto same-engine NoOps inserted directly
    before the instruction (sequencer order makes this semantically identical).
    Safe for a straight-line Tile program: waits only reference predecessors
    in the dependency DAG, so stalling the sequencer earlier cannot deadlock."""
    import concourse.mybir as mybir

    ctr = [0]
    for fn in nc.m.functions:
        for bb in fn.blocks:
            insts = bb.instructions
            out = []
            changed = False
            for inst in insts:
                si = getattr(inst, "sync_info", None)
                if si is not None and len(si.on_wait) > maxw:
                    waits = list(si.on_wait)
                    keep = waits[:maxw]
                    rest = waits[maxw:]
                    for i in range(0, len(rest), maxw):
                        n = mybir.InstNoOp(
                            name=f"waitsplit-{ctr[0]}", ins=[], outs=[]
                        )
                        ctr[0] += 1
                        n.engine = inst.engine
                        n.sync_info = mybir.SyncInfo(
                            on_wait=rest[i : i + maxw], on_update=[]
                        )
                        out.append(n)
                    inst.sync_info = mybir.SyncInfo(
                        on_wait=keep, on_update=list(si.on_update)
                    )
                    changed = True
                out.append(inst)
            if changed:
                bb.instructions = out
    return nc


def _build():
    import concourse.bass as bass
    import concourse.tile as tile
    import concourse.mybir as mybir
    from concourse.masks import make_identity, make_causal_mask

    f32 = mybir.dt.float32
    bf16 = mybir.dt.bfloat16

    nc = bass.Bass("TRN2", target_bir_lowering=False, debug=False, num_devices=NCORES)
    xT_ext = nc.declare_dram_parameter("xT", [E, S], bf16, isOutput=False)
    wq_ext = nc.declare_dram_parameter("wq", [E, HD], bf16, isOutput=False)
    wk_ext = nc.declare_dram_parameter("wk", [E, HD], bf16, isOutput=False)
    wv_ext = nc.declare_dram_parameter("wv", [E, HD], bf16, isOutput=False)
    wo_ext = nc.declare_dram_parameter("wo", [HD, E], bf16, isOutput=False)
    bo_ext = nc.declare_dram_parameter("bo", [P, E], f32, isOutput=False)
    out_ext = nc.declare_dram_parameter("out", [S, E], f32, isOutput=True)

    EXP = mybir.ActivationFunctionType.Exp

    with tile.TileContext(nc) as tc:
        with (
            tc.tile_pool(name="consts", bufs=1) as consts,
            tc.tile_pool(name="qt", bufs=NCH) as qt_pool,
            tc.tile_pool(name="kt", bufs=NCH) as kt_pool,
            tc.tile_pool(name="vp", bufs=NT) as v_pool,
            tc.tile_pool(name="ct", bufs=NCH) as ct_pool,
            tc.tile_pool(name="wo", bufs=NCH) as wo_pool,
            tc.tile_pool(name="outp", bufs=2) as out_pool,
            tc.tile_pool(name="sums", bufs=2) as sums_pool,
        ):
            identity = consts.tile([P, P], bf16, tag="ident")
            make_identity(nc, identity[:])
            mask = consts.tile([P, P], f32, tag="mask")
            make_causal_mask(nc, mask[:], mask_val=-1e9)
            bo_sb = consts.tile([P, E], f32, tag="bo")
            nc.sync.dma_start(out=bo_sb[:], in_=bo_ext[:])

            wo_sb = [wo_pool.tile([P, E], bf16, tag="wo", name=f"wo{_}") for _ in range(NCH)]
            for c in range(NCH):
                nc.sync.dma_start(out=wo_sb[c][:], in_=wo_ext[c * P : (c + 1) * P, :])

            qt_sb = [qt_pool.tile([P, S], bf16, tag="qt", name=f"qt{_}") for _ in range(NCH)]
            kt_sb = [kt_pool.tile([P, S], bf16, tag="kt", name=f"kt{_}") for _ in range(NCH)]
            v_sb = [v_pool.tile([P, HD], bf16, tag="v", name=f"v{_}") for _ in range(NT)]
            ct_sb = [ct_pool.tile([P, S], bf16, tag="ct", name=f"ct{_}") for _ in range(NCH)]

            # ---------- phase 1: QKV projections ----------
            with (
                tc.tile_pool(name="xt", bufs=NCH) as xt_pool,
                tc.tile_pool(name="w3", bufs=3 * NCH) as w3_pool,
                tc.tile_pool(name="psum1", bufs=3, space="PSUM") as psum1,
            ):
                xt_sb = [xt_pool.tile([P, S], bf16, tag="xt", name=f"xt{_}") for _ in range(NCH)]
                wq_sb = [w3_pool.tile([P, HD], bf16, tag="w3", name=f"wq{_}") for _ in range(NCH)]
                wk_sb = [w3_pool.tile([P, HD], bf16, tag="w3", name=f"wk{_}") for _ in range(NCH)]
                wv_sb = [w3_pool.tile([P, HD], bf16, tag="w3", name=f"wv{_}") for _ in range(NCH)]
                for c in range(NCH):
                    rows = slice(c * P, (c + 1) * P)
                    nc.sync.dma_start(out=xt_sb[c][:], in_=xT_ext[rows, :])
                    nc.sync.dma_start(out=wq_sb[c][:], in_=wq_ext[rows, :])
                    nc.sync.dma_start(out=wk_sb[c][:], in_=wk_ext[rows, :])
                    nc.sync.dma_start(out=wv_sb[c][:], in_=wv_ext[rows, :])

                for w_sb, dst in ((wq_sb, qt_sb), (wk_sb, kt_sb)):
                    for c in range(NCH):
                        ps = psum1.tile([P, S], f32, tag="mm")
                        for k in range(NCH):
                            for hf in range(2):
                                nc.tensor.matmul(
                                    ps[:, hf * 512 : (hf + 1) * 512],
                                    lhsT=w_sb[k][:, c * P : (c + 1) * P],
                                    rhs=xt_sb[k][:, hf * 512 : (hf + 1) * 512],
                                    start=(k == 0),
                                    stop=(k == NCH - 1),
                                )
                        nc.scalar.copy(out=dst[c][:], in_=ps[:])

                for t in range(NT):
                    ps = psum1.tile([P, HD], f32, tag="mm")
                    for k in range(NCH):
                        for hf in range(2):
                            nc.tensor.matmul(
                                ps[:, hf * 512 : (hf + 1) * 512],
                                lhsT=xt_sb[k][:, t * P : (t + 1) * P],
                                rhs=wv_sb[k][:, hf * 512 : (hf + 1) * 512],
                                start=(k == 0),
                                stop=(k == NCH - 1),
                            )
                    nc.scalar.copy(out=v_sb[t][:], in_=ps[:])

            # ---------- phase 2: attention per head ----------
            with (
                tc.tile_pool(name="attn", bufs=3) as attn_pool,
                tc.tile_pool(name="attnT", bufs=2) as attnT_pool,
                tc.tile_pool(name="diag", bufs=3) as diag_pool,
                tc.tile_pool(name="psum_sc", bufs=2, space="PSUM") as psum_sc,
                tc.tile_pool(name="psum_pt", bufs=2, space="PSUM") as psum_pt,
                tc.tile_pool(name="psum_ot", bufs=1, space="PSUM") as psum_ot,
            ):
                for h in range(H):
                    cq = h // 2
                    off = (h % 2) * D
                    qt_h = qt_sb[cq][off : off + D, :]
                    kt_h = kt_sb[cq][off : off + D, :]
                    sums_h = sums_pool.tile([P, NT], f32, tag="sums")
                    r_h = sums_pool.tile([P, NT], f32, tag="r")
                    attnT_tiles = []
                    for t in range(NT):
                        k_len = (t + 1) * P
                        sc = psum_sc.tile([P, S], f32, tag="sc")
                        for j in range((k_len + 511) // 512):
                            c0, c1 = j * 512, min(k_len, (j + 1) * 512)
                            nc.tensor.matmul(
                                sc[:, c0:c1],
                                lhsT=qt_h[:, t * P : (t + 1) * P],
                                rhs=kt_h[:, c0:c1],
                                start=True,
                                stop=True,
                            )
                        nc.vector.tensor_add(
                            sc[:, t * P : k_len], sc[:, t * P : k_len], mask[:]
                        )
                        attn = attn_pool.tile([P, S], bf16, tag="attn")
                        nc.scalar.activation(
                            attn[:, :k_len],
                            sc[:, :k_len],
                            EXP,
                            bias=0.0,
                            scale=float(SCALE),
                            accum_out=sums_h[:, t : t + 1],
                        )
                        nc.vector.reciprocal(r_h[:, t : t + 1], sums_h[:, t : t + 1])
                        dg = diag_pool.tile([P, P], bf16, tag="diag")
                        nc.vector.tensor_scalar_mul(
                            dg[:], identity[:], r_h[:, t : t + 1]
                        )
                        aT = attnT_pool.tile([P, S], bf16, tag=f"aT{t}")
                        attnT_tiles.append(aT)
                        for g0 in range(0, t + 1, 4):
                            g1 = min(g0 + 4, t + 1)
                            pt = psum_pt.tile([P, 512], f32, tag="pt")
                            for kc in range(g0, g1):
                                nc.tensor.matmul(
                                    pt[:, (kc - g0) * P : (kc - g0 + 1) * P],
                                    lhsT=attn[:, kc * P : (kc + 1) * P],
                                    rhs=dg[:],
                                    start=True,
                                    stop=True,
                                )
                            nc.vector.tensor_copy(
                                aT[:, g0 * P : g1 * P], pt[:, : (g1 - g0) * P]
                            )
                    # t-outer so each PSUM accumulation group is contiguous:
                    # a start=True matmul invalidates has_written for its whole
                    # bank, so groups sharing a bank must not interleave.
                    ot = psum_ot.tile([D, S], f32, tag="ot")
                    for t in range(NT):
                        for kc in range(t + 1):
                            nc.tensor.matmul(
                                ot[:, t * P : (t + 1) * P],
                                lhsT=v_sb[kc][:, h * D : (h + 1) * D],
                                rhs=attnT_tiles[t][:, kc * P : (kc + 1) * P],
                                start=(kc == 0),
                                stop=(kc == t),
                            )
                    nc.scalar.copy(out=ct_sb[cq][off : off + D, :], in_=ot[:])

            # ---------- phase 3: output projection + bias ----------
            with tc.tile_pool(name="psum3", bufs=2, space="PSUM") as psum3:
                for t in range(NT):
                    op = psum3.tile([P, E], f32, tag="mm3")
                    for c in range(NCH):
                        for hf in range(2):
                            nc.tensor.matmul(
                                op[:, hf * 512 : (hf + 1) * 512],
                                lhsT=ct_sb[c][:, t * P : (t + 1) * P],
                                rhs=wo_sb[c][:, hf * 512 : (hf + 1) * 512],
                                start=(c == 0),
                                stop=(c == NCH - 1),
                            )
                    osb = out_pool.tile([P, E], f32, tag="out")
                    nc.vector.tensor_add(osb[:], op[:], bo_sb[:])
                    nc.sync.dma_start(
                        out=out_ext[t * P : (t + 1) * P, :], in_=osb[:]
                    )

    return _split_waits(nc)


def _get_graph():
    if "nc" not in _graph_cache:
        _patch_tile_drain()
        _graph_cache["nc"] = _build()
    return _graph_cache["nc"]


def _prep_inputs(x, Wq, Wk, Wv, Wo, bo):
    xT = np.ascontiguousarray(np.transpose(np.asarray(x, np.float32), (0, 2, 1)))
    xT = xT.astype(BF16)
    wq = np.ascontiguousarray(
        np.asarray(Wq, np.float32).transpose(1, 0, 2).reshape(E, HD)
    ).astype(BF16)
    wk = np.ascontiguousarray(
        np.asarray(Wk, np.float32).transpose(1, 0, 2).reshape(E, HD)
    ).astype(BF16)
    wv = np.ascontiguousarray(
        np.asarray(Wv, np.float32).transpose(1, 0, 2).reshape(E, HD)
    ).astype(BF16)
    wo = np.ascontiguousarray(np.asarray(Wo, np.float32)).astype(BF16)
    bo_t = np.ascontiguousarray(
        np.tile(np.asarray(bo, np.float32)[None, :], (P, 1))
    )
    return [
        dict(
            xT=np.ascontiguousarray(xT[b]),
            wq=wq,
            wk=wk,
            wv=wv,
            wo=wo,
            bo=bo_t,
        )
        for b in range(B)
    ]


def _run(in_maps, **kw):
    from concourse.bass_utils import run_bass_kernel_spmd

    nc = _get_graph()
    return run_bass_kernel_spmd(nc, in_maps, core_ids=list(range(NCORES)), **kw)


def kernel(x, Wq, Wk, Wv, Wo, bo):
    res = _run(_prep_inputs(x, Wq, Wk, Wv, Wo, bo))
    return np.stack(
        [np.asarray(res.results[b]["out"], np.float32) for b in range(B)], axis=0
    )
